# revision 2
# baseline (speedup 1.0000x reference)
"""Trainium2 Bass kernel for nn_Dist_Conv2D (Chebyshev-distance conv).

out[b,o,h,w] = max_{c,kh,kw} |x_pad[b,c,h+kh,w+kw] - weights[o,c,kh,kw]| + bias[o]

Strategy:
- Data-parallel over batch: 16 batches -> 8 cores x 2 batches.
- Host prep: pad x to 58x58, channels-last layout [b, hp, wp, c]. Output
  positions are indexed local = h*58 + w' (w' in [0,58), the 2 extra
  "halo" columns are computed and discarded) so that consecutive
  positions are unit-stride in the padded image -> the im2col patch load
  is a single strided DMA per 128-position tile.
- Device: one fused custom DVE instruction per (128-position tile, output
  channel): accum[p] = max(seed, max_d |x[p,d] - w[o,d]|), d = (kh,kw,c),
  D = 576. Weights are broadcast across partitions once per o (amortized
  over all 52 tiles); x tiles stay resident in SBUF.
"""

import numpy as np

import concourse.bacc as bacc
import concourse.mybir as mybir
from concourse.tile import TileContext
from concourse.bass_utils import run_bass_kernel_spmd

# ---------------------------------------------------------------------------
# Custom DVE op: out = |in0 - in1|, accum_out = max(s0, max_k out[k]).
# Registered into concourse.dve_ops at import time; the per-NEFF DVE table
# is generated client-side from dve_ops.OPS, so runtime registration is
# visible to the compile.
# ---------------------------------------------------------------------------
from concourse import dve_ops as _dve_ops
from concourse.dve_ops import DveOp as _DveOp
from concourse.dve_spec import (
    Spec as _Spec,
    Src0 as _Src0,
    Src1 as _Src1,
    C0 as _C0,
    maxx as _maxx,
    lower as _dve_lower,
    _has_src1,
)
from concourse.dve_uop import DveOpSpec as _DveOpSpec


def _ref_cheb(in0, in1, s0, s1, imm2):
    p = in0.shape[0]
    a = in0.astype(np.float32).reshape(p, -1)
    b = in1.astype(np.float32).reshape(p, -1)
    body = np.abs(a - b).astype(np.float32)
    seed = np.asarray(s0, np.float32).reshape(-1, 1)
    acc = np.maximum(np.max(body, axis=-1, keepdims=True), seed)
    return body.reshape(in0.shape), acc


_CHEB_SPEC = _Spec(
    body=_maxx(_Src0 - _Src1, _Src1 - _Src0),
    accum=_maxx,
    accum_init=_C0,
    reference=_ref_cheb,
)
_CHEB_NAME = "CHEB_ABSDIFF_MAX_ANT"


def _register_cheb_op() -> _DveOp:
    for op in _dve_ops.OPS:
        if op.name == _CHEB_NAME:
            return op
    row = _dve_ops._CUSTOM_DVE_ROW_BASE + len(_dve_ops.OPS)
    assert row < 0x20
    shas = {}
    for ver in ("v3", "v4"):
        s = _DveOpSpec(
            name=_CHEB_NAME,
            opcode=row,
            uops=_dve_lower(_CHEB_SPEC, ver=ver),
            rd1_en=_has_src1(_CHEB_SPEC),
        )
        shas[ver] = s.sha(ver)
    op = _DveOp(_CHEB_NAME, _CHEB_SPEC, subdim=False, uops_sha=shas)
    _dve_ops.OPS.append(op)
    _dve_ops.CUSTOM_DVE_SPECS[_CHEB_NAME] = _CHEB_SPEC
    _dve_ops._SUB_OPCODE_FOR_NAME[_CHEB_NAME] = row
    return op


CHEB_OP = _register_cheb_op()

# ---------------------------------------------------------------------------
# Problem geometry (hardcoded for this problem instance).
# ---------------------------------------------------------------------------
B, CIN, H, W = 16, 64, 56, 56
COUT, K = 128, 3
PADL = 1  # PADDING=2 split 1/1
HP, WP = H + 2, W + 2  # 58 x 58 padded image
D = CIN * K * K  # 576, patch feature dim, ordered (kh, kw, c)
NCORES = 8
B_PER = B // NCORES  # 2 batches per core
POS_PER_BATCH = H * WP  # 3248 positions incl. 2 halo columns per row
P = 128  # partitions
TILES_PER_BATCH = -(-POS_PER_BATCH // P)  # 26 (last tile 48 pos of slop)
NTILES = B_PER * TILES_PER_BATCH  # 52 position tiles per core
XS_IMG = HP * WP * CIN  # 215296 elements per padded channels-last image
# max read: b=B_PER-1, local=TILES_PER_BATCH*P-1, kh=kw=2, c=63
_XS_MAX = (B_PER - 1) * XS_IMG + (TILES_PER_BATCH * P - 1 + 2 * WP + 2) * CIN + CIN
XS_SIZE = max(B_PER * XS_IMG, _XS_MAX) + 256

_CACHE = {}


def _build_program():
    if "nc" in _CACHE:
        return _CACHE["nc"]
    nc = bacc.Bacc("TRN2", num_devices=NCORES)
    xs_ext = nc.declare_dram_parameter("xs", [XS_SIZE], mybir.dt.float32, isOutput=False)
    wr_ext = nc.declare_dram_parameter("wr", [COUT, D], mybir.dt.float32, isOutput=False)
    bias_ext = nc.declare_dram_parameter("bias", [1, COUT], mybir.dt.float32, isOutput=False)
    out_ext = nc.declare_dram_parameter(
        "out", [NTILES * P, COUT], mybir.dt.float32, isOutput=True
    )

    ap_cls = type(xs_ext[:].ap)

    with TileContext(nc) as tc:
        with tc.tile_pool(name="sbuf", bufs=1) as pool:
            xbig = pool.tile([P, NTILES * D], mybir.dt.float32)
            # Patch loads: one strided DMA per (batch, tile).
            for b in range(B_PER):
                for t in range(TILES_PER_BATCH):
                    idx = b * TILES_PER_BATCH + t
                    src = xs_ext[:].copy()
                    src.offset = b * XS_IMG + t * P * CIN
                    src.ap = ap_cls(
                        [[CIN, P], [WP * CIN, K], [CIN, K], [1, CIN]]
                    )
                    nc.sync.dma_start(xbig[:, idx * D : (idx + 1) * D], src)

            acc = pool.tile([P, NTILES * COUT], mybir.dt.float32)
            dummy = pool.tile([P, 1], mybir.dt.float32)
            bias_b = pool.tile([P, COUT], mybir.dt.float32)
            nc.sync.dma_start(bias_b[:], bias_ext[0:1, :].broadcast_to([P, COUT]))

            for o in range(COUT):
                wb = pool.tile([P, D], mybir.dt.float32, tag=f"wb{o % 2}")
                nc.sync.dma_start(wb[:], wr_ext[o : o + 1, :].broadcast_to([P, D]))
                for idx in range(NTILES):
                    col = idx * COUT + o
                    nc.vector._custom_dve(
                        CHEB_OP,
                        out=dummy[:].broadcast_to([P, D]),
                        in0=xbig[:, idx * D : (idx + 1) * D],
                        in1=wb[:],
                        s0=0.0,
                        accum_out=acc[:, col : col + 1],
                    )

            # bias add (bias varies along the o (free) axis)
            for idx in range(NTILES):
                nc.vector.tensor_tensor(
                    acc[:, idx * COUT : (idx + 1) * COUT],
                    acc[:, idx * COUT : (idx + 1) * COUT],
                    bias_b[:],
                    mybir.AluOpType.add,
                )

            # out[(idx, p), o] = acc[p, idx*COUT + o]
            nc.sync.dma_start(
                out_ext[:].rearrange("(t p) o -> p t o", p=P),
                acc[:].rearrange("p (t o) -> p t o", o=COUT),
            )

    nc.compile()
    _CACHE["nc"] = nc
    return nc


def _prep_inputs(x, weights, bias):
    xp = np.pad(
        x.astype(np.float32, copy=False),
        ((0, 0), (0, 0), (PADL, PADL), (PADL, PADL)),
    )
    # channels-last [b, hp, wp, c]
    xcl = np.ascontiguousarray(xp.transpose(0, 2, 3, 1))
    wr = np.ascontiguousarray(
        weights.astype(np.float32, copy=False).transpose(0, 2, 3, 1).reshape(COUT, D)
    )
    bias_row = np.ascontiguousarray(
        bias.astype(np.float32, copy=False).reshape(1, COUT)
    )
    in_maps = []
    for core in range(NCORES):
        sl = xcl[core * B_PER : (core + 1) * B_PER].reshape(-1)
        xs = np.zeros(XS_SIZE, dtype=np.float32)
        xs[: sl.size] = sl
        in_maps.append({"xs": xs, "wr": wr, "bias": bias_row})
    return in_maps


def _unshard(results):
    outs = []
    for core in range(NCORES):
        r = results[core]["out"]  # [NTILES*P, COUT]
        r = r.reshape(B_PER, TILES_PER_BATCH * P, COUT)[:, :POS_PER_BATCH, :]
        r = r.reshape(B_PER, H, WP, COUT)[:, :, :W, :]
        outs.append(r.transpose(0, 3, 1, 2))  # [B_PER, COUT, H, W]
    return np.concatenate(outs, axis=0)


def kernel(x, weights, bias):
    nc = _build_program()
    in_maps = _prep_inputs(np.asarray(x), np.asarray(weights), np.asarray(bias))
    res = run_bass_kernel_spmd(nc, in_maps, core_ids=list(range(NCORES)))
    return _unshard(res.results).astype(np.float32)


# revision 4
# speedup vs baseline: 775.2006x; 775.2006x over previous
"""Trainium2 Bass kernel for nn_Dist_Conv2D (Chebyshev-distance conv).

out[b,o,h,w] = max_{c,kh,kw} |x_pad[b,c,h+kh,w+kw] - weights[o,c,kh,kw]| + bias[o]
x: [16,64,56,56] f32, weights: [128,64,3,3] f32, bias: [128,1,1] f32,
K=3, stride 1, pad 1/1 -> out [16,128,56,56] f32.

Strategy (8 NeuronCores, data-parallel over batch, 2 images per core):

- Host prep: pad x to 58x58 and switch to channels-last [b, hp, wp, c],
  cast to bf16. Output positions are indexed local = h*58 + w' with
  w' in [0,58) — the two halo columns are computed and discarded — so
  consecutive positions are unit-stride in the padded image and the
  im2col patch load for a 128-position tile is a single strided DMA
  (partition stride = C, free dims = (kh, kw, c) with c contiguous).

- Device: one fused custom DVE instruction per (128-position tile,
  output channel): the instruction body computes a running (prefix)
  maximum of |x - w| over the 576-element patch via a scan recurrence
  in the vector engine's 8-stage datapath (ABSOLUTE_DIFF + MAX with
  CURR_ALU_OUT feedback). The dst access pattern [[0,288],[1,2]]
  overwrites just two addresses alternately, so the final write — the
  complete max over the patch — lands at address 1. No reduce
  instruction, no accumulator readback: exactly one DVE pass per
  element.

- A hand-authored 2x_1p micro-op program (registered at perf slot +1,
  instruction perf_max=1) processes two packed bf16 elements per cycle:
  stage0 |lo|, stage1 |hi|, stage2 pairwise max, stage3 running max.
  All streamed operands are bf16 unit-stride so the RTL engages 2x;
  measured ~2.0x over the fp32 1x variant (2.66 ms vs 5.45 ms per
  kernel on hardware, 8 cores).

- Weights are broadcast across partitions once per output channel
  (amortized over all 52 position tiles); x tiles stay resident in
  SBUF; bias is added on-device in one tensor_tensor; one gather DMA
  writes [positions, channels] to DRAM; host drops halo columns and
  transposes to NCHW.
"""

import numpy as np
import ml_dtypes

import concourse.bacc as bacc
import concourse.mybir as mybir
from concourse.tile import TileContext
from concourse.bass_utils import run_bass_kernel_spmd

from concourse import dve_ops as _dve_ops
from concourse.dve_ops import DveOp as _DveOp
from concourse.dve_spec import (
    Spec as _Spec,
    Src0 as _Src0,
    Src1 as _Src1,
    Bin as _Bin,
    AluOp as _SpecAluOp,
    scan as _scan,
    lower as _dve_lower,
)
from concourse.dve_uop import (
    UopConfig,
    AluOp,
    AluInp,
    InpSel,
    OutSel,
    OutPath,
    Trigger,
    DveOpSpec,
    ENABLE,
)

# ---------------------------------------------------------------------------
# Problem geometry (hardcoded for this problem instance).
# ---------------------------------------------------------------------------
B, CIN, H, W = 16, 64, 56, 56
COUT, K = 128, 3
PADL = 1  # PADDING=2 split 1/1
HP, WP = H + 2, W + 2  # 58 x 58 padded image
D = CIN * K * K  # 576, patch feature dim, ordered (kh, kw, c)
NCORES = 8
B_PER = B // NCORES  # 2 batches per core
POS_PER_BATCH = H * WP  # 3248 positions incl. 2 halo columns per row
P = 128  # partitions
TILES_PER_BATCH = -(-POS_PER_BATCH // P)  # 26
NTILES = B_PER * TILES_PER_BATCH  # 52 position tiles per core
XS_IMG = HP * WP * CIN  # elements per padded channels-last image
_XS_MAX = (B_PER - 1) * XS_IMG + (TILES_PER_BATCH * P - 1 + 2 * WP + 2) * CIN + CIN
XS_SIZE = max(B_PER * XS_IMG, _XS_MAX) + 256
BF16 = mybir.dt.bfloat16

# ---------------------------------------------------------------------------
# Custom DVE op: body = prefix-max of |in0 - in1| along the free dim.
# Registered into concourse.dve_ops at import time (the per-NEFF DVE table
# is generated client-side from dve_ops.OPS, so runtime registration is
# visible to the compile).
# ---------------------------------------------------------------------------


def _ref_scan(in0, in1, s0, s1, imm2):
    p = in0.shape[0]
    a = in0.astype(np.float32).reshape(p, -1)
    b = in1.astype(np.float32).reshape(p, -1)
    return np.maximum.accumulate(np.abs(a - b), axis=1).reshape(in0.shape)


_SCAN_SPEC = _Spec(
    body=_scan(_SpecAluOp.MAX, _Bin(_SpecAluOp.ABSOLUTE_DIFF, _Src0, _Src1)),
    reference=_ref_scan,
)
_SCAN_NAME = "CHEB_SCANMAX_ANT"


def _mk_scan_2x_uops():
    # crossbar lanes (lane k>=1 feeds stage0's PREV_DELAY_{k-1}):
    #   1=SRC_0, 2=SRC_1, 3=MAX_NEG, 4=SRC_0_HI, 5=SRC_1_HI
    def wire(u):
        u.enable_input(InpSel.SRC_0, 1)
        u.enable_input(InpSel.SRC_1, 2)
        u.enable_input(InpSel.MAX_NEG, 3)
        u.enable_input(InpSel.SRC_0_HI, 4)
        u.enable_input(InpSel.SRC_1_HI, 5)
        return u

    # seed: one cycle, no stream consumed; initialise stage3's CURR_ALU_OUT
    # (the scan recurrence register) with -FLT_MAX.
    seed = wire(UopConfig())
    seed.repeat_count = 1
    seed.trigger = (Trigger.COUNT, Trigger.NONE, Trigger.NONE)
    seed.next_uop = (1, 0, 0)
    for st in range(8):
        dp = seed.datapath_config[st]
        if st < 3:
            dp.pass_through_alu()
            dp.pass_through_delay(2)
        elif st == 3:
            dp.enable_alu(AluOp.BYPASS, AluInp.PREV_DELAY_2, AluInp.PREV_DELAY_2)
        else:
            dp.pass_through_alu()

    # steady: two packed elements per cycle.
    st_ = wire(UopConfig())
    st_.require_inp0 = ENABLE
    st_.require_inp1 = ENABLE
    st_.trigger = (Trigger.SRC_TENSOR_DONE, Trigger.NONE, Trigger.NONE)
    st_.next_uop = (0, 0, 0)
    st_.enable_output(OutSel.DELAY_0, OutPath.WR0_LO)  # |lo| (discarded)
    st_.enable_output(OutSel.ALU_OUT, OutPath.WR0_HI)  # running max
    dps = st_.datapath_config
    dps[0].enable_alu(AluOp.ABSOLUTE_DIFF, AluInp.PREV_DELAY_0, AluInp.PREV_DELAY_1)
    dps[0].pass_through_delay(3, 4)
    dps[1].enable_alu(AluOp.ABSOLUTE_DIFF, AluInp.PREV_DELAY_3, AluInp.PREV_DELAY_4)
    dps[1].enable_delay_from_src(AluInp.PREV_ALU_OUT, 0)  # lane0 <- |lo|
    dps[2].enable_alu(AluOp.MAX, AluInp.PREV_ALU_OUT, AluInp.PREV_DELAY_0)
    dps[2].pass_through_delay(0)
    dps[3].enable_alu(AluOp.MAX, AluInp.CURR_ALU_OUT, AluInp.PREV_ALU_OUT)
    dps[3].pass_through_delay(0)
    for st in range(4, 8):
        dps[st].pass_through_alu()
        dps[st].pass_through_delay(0)
    return [seed, st_]


class _ScanOp(_DveOp):
    """DveOp whose compile() attaches the hand-written 2x uops."""

    def compile(self, ver):
        key = (self.name, ver)
        cached = _dve_ops._COMPILE_CACHE.get(key)
        if cached is not None:
            return cached
        spec = DveOpSpec(
            name=self.name,
            opcode=_dve_ops.get_dve_sub_opcode(self.name),
            uops=_dve_lower(self.spec, ver=ver),
            rd1_en=True,
            uops_2x=_mk_scan_2x_uops(),
            perf_max=1,
        )
        _dve_ops._COMPILE_CACHE[key] = spec
        return spec


def _register() -> _DveOp:
    for op in _dve_ops.OPS:
        if op.name == _SCAN_NAME:
            return op
    row = _dve_ops._CUSTOM_DVE_ROW_BASE + len(_dve_ops.OPS)
    assert row < 0x20
    op = _ScanOp(_SCAN_NAME, _SCAN_SPEC, subdim=False, uops_sha={})
    _dve_ops.OPS.append(op)
    _dve_ops.CUSTOM_DVE_SPECS[_SCAN_NAME] = _SCAN_SPEC
    _dve_ops._SUB_OPCODE_FOR_NAME[_SCAN_NAME] = row
    return op


SCAN_OP = _register()

_CACHE = {}


def _build_program(loop_n=None, perf_max=1):
    key = ("nc", loop_n, perf_max)
    if key in _CACHE:
        return _CACHE[key]
    nc = bacc.Bacc("TRN2", num_devices=NCORES)
    xs_ext = nc.declare_dram_parameter("xs", [XS_SIZE], BF16, isOutput=False)
    wr_ext = nc.declare_dram_parameter("wr", [COUT, D], BF16, isOutput=False)
    bias_ext = nc.declare_dram_parameter("bias", [1, COUT], mybir.dt.float32, isOutput=False)
    out_ext = nc.declare_dram_parameter(
        "out", [NTILES * P, COUT], mybir.dt.float32, isOutput=True
    )
    ap_cls = type(xs_ext[:].ap)

    with TileContext(nc) as tc:
        with tc.tile_pool(name="sbuf", bufs=1) as pool:
            from contextlib import nullcontext

            loop_cm = tc.For_i(0, loop_n, 1) if loop_n else nullcontext()
            with loop_cm:
                xbig = pool.tile([P, NTILES * D], BF16)
                # im2col patch loads: one strided DMA per (batch, tile)
                for b in range(B_PER):
                    for t in range(TILES_PER_BATCH):
                        idx = b * TILES_PER_BATCH + t
                        src = xs_ext[:].copy()
                        src.offset = b * XS_IMG + t * P * CIN
                        src.ap = ap_cls([[CIN, P], [WP * CIN, K], [CIN, K], [1, CIN]])
                        nc.sync.dma_start(xbig[:, idx * D : (idx + 1) * D], src)

                dstw = 2 * NTILES * COUT
                dst_big = pool.tile([P, dstw], BF16)
                acc = pool.tile([P, NTILES * COUT], mybir.dt.float32)
                bias_b = pool.tile([P, COUT], mybir.dt.float32)
                nc.sync.dma_start(bias_b[:], bias_ext[0:1, :].broadcast_to([P, COUT]))

                for o in range(COUT):
                    wb = pool.tile([P, D], BF16, tag=f"wb{o % 2}")
                    nc.sync.dma_start(wb[:], wr_ext[o : o + 1, :].broadcast_to([P, D]))
                    for idx in range(NTILES):
                        col = idx * COUT + o
                        dsl = dst_big[:].copy()
                        dsl.offset = dst_big[:].offset + 2 * col
                        dsl.ap = ap_cls([[dstw, P], [0, D // 2], [1, 2]])
                        r = nc.vector._custom_dve(
                            SCAN_OP,
                            out=dsl,
                            in0=xbig[:, idx * D : (idx + 1) * D],
                            in1=wb[:],
                            accum_out=None,
                        )
                        r.ins.perf_max = perf_max

                # acc = dst_big[:, 1::2] + bias (bias repeats per tile)
                din = dst_big[:].copy()
                din.offset = dst_big[:].offset + 1
                din.ap = ap_cls([[dstw, P], [2 * COUT, NTILES], [2, COUT]])
                bin_ = bias_b[:].copy()
                bin_.ap = ap_cls([[COUT, P], [0, NTILES], [1, COUT]])
                nc.vector.tensor_tensor(
                    acc[:].rearrange("p (t o) -> p t o", o=COUT),
                    din,
                    bin_,
                    mybir.AluOpType.add,
                )

                # out[(t,p), o] = acc[p, t*COUT + o]
                nc.sync.dma_start(
                    out_ext[:].rearrange("(t p) o -> p t o", p=P),
                    acc[:].rearrange("p (t o) -> p t o", o=COUT),
                )

    nc.compile()
    _CACHE[key] = nc
    return nc


def _prep_inputs(x, weights, bias):
    xp = np.pad(
        x.astype(np.float32, copy=False),
        ((0, 0), (0, 0), (PADL, PADL), (PADL, PADL)),
    )
    xcl = np.ascontiguousarray(xp.transpose(0, 2, 3, 1)).astype(ml_dtypes.bfloat16)
    wr = np.ascontiguousarray(
        weights.astype(np.float32, copy=False).transpose(0, 2, 3, 1).reshape(COUT, D)
    ).astype(ml_dtypes.bfloat16)
    bias_row = np.ascontiguousarray(bias.astype(np.float32, copy=False).reshape(1, COUT))
    in_maps = []
    for core in range(NCORES):
        sl = xcl[core * B_PER : (core + 1) * B_PER].reshape(-1)
        xs = np.zeros(XS_SIZE, dtype=ml_dtypes.bfloat16)
        xs[: sl.size] = sl
        in_maps.append({"xs": xs, "wr": wr, "bias": bias_row})
    return in_maps


def _unshard(results):
    outs = []
    for core in range(NCORES):
        r = results[core]["out"]  # [NTILES*P, COUT]
        r = r.reshape(B_PER, TILES_PER_BATCH * P, COUT)[:, :POS_PER_BATCH, :]
        r = r.reshape(B_PER, H, WP, COUT)[:, :, :W, :]
        outs.append(r.transpose(0, 3, 1, 2))  # [B_PER, COUT, H, W]
    return np.concatenate(outs, axis=0)


def kernel(x, weights, bias):
    nc = _build_program()
    in_maps = _prep_inputs(np.asarray(x), np.asarray(weights), np.asarray(bias))
    res = run_bass_kernel_spmd(nc, in_maps, core_ids=list(range(NCORES)))
    return _unshard(res.results).astype(np.float32)


# revision 5
# speedup vs baseline: 917.1596x; 1.1831x over previous
"""Trainium2 Bass kernel for nn_Dist_Conv2D (Chebyshev-distance conv).

out[b,o,h,w] = max_{c,kh,kw} |x_pad[b,c,h+kh,w+kw] - weights[o,c,kh,kw]| + bias[o]
x: [16,64,56,56] f32, weights: [128,64,3,3] f32, bias: [128,1,1] f32,
K=3, stride 1, pad 1/1 -> out [16,128,56,56] f32.

Strategy (8 NeuronCores, data-parallel over batch, 2 images per core):

- Host prep: pad x to 58x58, channels-last [b, hp, wp, c], cast bf16.
  Output positions are indexed local = h*58 + w' with w' in [0,58) — the
  two halo columns are computed and discarded — so consecutive positions
  are unit-stride in the padded image and the im2col patch load for a
  128-position tile is a single strided DMA.

- Device: one fused custom DVE instruction per (128-position tile, group
  of 8 output channels). The instruction streams [P, S=8 pages, 576]
  where in0 is the x patch tile with page stride 0 and in1 holds 8
  partition-broadcast weight rows. The body computes a running (prefix)
  maximum of |x - w| via a scan recurrence (ABSOLUTE_DIFF + MAX with
  CURR_ALU_OUT feedback); a 3-state uop FSM (seed / steady / reseed)
  restarts the recurrence at each SUB_DIM_DONE page boundary. Each
  page's final element is that (tile, o)'s complete max; the otherwise
  idle Scalar engine gathers the 8 values per instruction into the fp32
  accumulator while the DVE streams on. One DVE pass per element, no
  reduce instructions.

- A hand-authored 2x_1p micro-op program (perf slot +1, instruction
  perf_max=1) processes two packed bf16 elements per cycle: stage0 |lo|,
  stage1 |hi| via the SRC_*_HI crossbar lanes, stage2 pair max, stage3
  recurrence. All streamed operands are bf16 unit-stride innermost so
  the RTL engages 2x. Measured on HW (loop-delta method): 2.36 ms per
  kernel vs 5.45 ms for the fp32 1x un-paged variant.

- Weights are broadcast across partitions once per 8-channel group;
  x tiles stay resident in SBUF; bias is added on-device; one gather
  DMA writes [positions, channels]; host drops halo columns and
  transposes to NCHW.
"""

import numpy as np
import ml_dtypes

import concourse.bacc as bacc
import concourse.mybir as mybir
from concourse.tile import TileContext
from concourse.bass_utils import run_bass_kernel_spmd

from concourse import dve_ops as _dve_ops
from concourse.dve_ops import DveOp as _DveOp
from concourse.dve_spec import (
    Spec as _Spec,
    Src0 as _Src0,
    Src1 as _Src1,
    Bin as _Bin,
    AluOp as _SpecAluOp,
    scan as _scan,
)
from concourse.dve_uop import (
    UopConfig,
    AluOp,
    AluInp,
    InpSel,
    OutSel,
    OutPath,
    Trigger,
    DveOpSpec,
    ENABLE,
)

# ---------------------------------------------------------------------------
# Problem geometry (hardcoded for this problem instance).
# ---------------------------------------------------------------------------
B, CIN, H, W = 16, 64, 56, 56
COUT, K = 128, 3
PADL = 1  # PADDING=2 split 1/1
HP, WP = H + 2, W + 2  # 58 x 58 padded image
D = CIN * K * K  # 576, patch feature dim, ordered (kh, kw, c)
NCORES = 8
B_PER = B // NCORES  # 2 batches per core
POS_PER_BATCH = H * WP  # 3248 positions incl. 2 halo columns per row
P = 128  # partitions
TILES_PER_BATCH = -(-POS_PER_BATCH // P)  # 26
NTILES = B_PER * TILES_PER_BATCH  # 52 position tiles per core
XS_IMG = HP * WP * CIN  # elements per padded channels-last image
_XS_MAX = (B_PER - 1) * XS_IMG + (TILES_PER_BATCH * P - 1 + 2 * WP + 2) * CIN + CIN
XS_SIZE = max(B_PER * XS_IMG, _XS_MAX) + 256
BF16 = mybir.dt.bfloat16
S = 8  # output channels (pages) per DVE instruction

# ---------------------------------------------------------------------------
# Custom DVE op: per-page prefix-max of |in0 - in1| over [P, S, N] streams.
# Registered into concourse.dve_ops at import time (the per-NEFF DVE table
# is generated client-side from dve_ops.OPS, so runtime registration is
# visible to the compile).
# ---------------------------------------------------------------------------


def _ref_paged(in0, in1, s0, s1, imm2):
    a = in0.astype(np.float32)
    b = in1.astype(np.float32)
    return np.maximum.accumulate(np.abs(a - b), axis=-1)


_PAGED_SPEC = _Spec(
    body=_scan(_SpecAluOp.MAX, _Bin(_SpecAluOp.ABSOLUTE_DIFF, _Src0, _Src1)),
    reference=_ref_paged,
)
_PAGED_NAME = "CHEB_PAGED_SCANMAX_ANT"


def _wire(u, hi):
    # crossbar lanes (lane k>=1 feeds stage0's PREV_DELAY_{k-1})
    u.enable_input(InpSel.SRC_0, 1)
    u.enable_input(InpSel.SRC_1, 2)
    u.enable_input(InpSel.MAX_NEG, 3)
    if hi:
        u.enable_input(InpSel.SRC_0_HI, 4)
        u.enable_input(InpSel.SRC_1_HI, 5)
    return u


def _mk_1x_uops():
    # scan recurrence register = stage 1's CURR_ALU_OUT flop
    seed = _wire(UopConfig(), hi=False)
    seed.repeat_count = 1
    seed.trigger = (Trigger.COUNT, Trigger.NONE, Trigger.NONE)
    seed.next_uop = (1, 0, 0)
    seed.datapath_config[0].pass_through_alu()
    seed.datapath_config[0].pass_through_delay(2)
    seed.datapath_config[1].enable_alu(
        AluOp.BYPASS, AluInp.PREV_DELAY_2, AluInp.PREV_DELAY_2
    )
    for st in range(2, 8):
        seed.datapath_config[st].pass_through_alu()

    def work(reseed):
        u = _wire(UopConfig(), hi=False)
        u.require_inp0 = ENABLE
        u.require_inp1 = ENABLE
        u.enable_output(OutSel.ALU_OUT, OutPath.WR0_LO)
        dps = u.datapath_config
        dps[0].enable_alu(
            AluOp.ABSOLUTE_DIFF, AluInp.PREV_DELAY_0, AluInp.PREV_DELAY_1
        )
        if reseed:
            # first element of a new page: recurrence <- |elem|
            dps[1].enable_alu(AluOp.BYPASS, AluInp.PREV_ALU_OUT, AluInp.PREV_ALU_OUT)
            u.repeat_count = 1
            u.trigger = (Trigger.COUNT, Trigger.NONE, Trigger.NONE)
            u.next_uop = (1, 0, 0)
        else:
            dps[1].enable_alu(AluOp.MAX, AluInp.CURR_ALU_OUT, AluInp.PREV_ALU_OUT)
            u.trigger = (Trigger.SRC_TENSOR_DONE, Trigger.SUB_DIM_DONE, Trigger.NONE)
            u.next_uop = (0, 2, 0)
        for st in range(2, 8):
            dps[st].pass_through_alu()
        return u

    return [seed, work(False), work(True)]


def _mk_2x_uops():
    seed = _wire(UopConfig(), hi=True)
    seed.repeat_count = 1
    seed.trigger = (Trigger.COUNT, Trigger.NONE, Trigger.NONE)
    seed.next_uop = (1, 0, 0)
    for st in range(8):
        dp = seed.datapath_config[st]
        if st < 3:
            dp.pass_through_alu()
            dp.pass_through_delay(2)
        elif st == 3:
            dp.enable_alu(AluOp.BYPASS, AluInp.PREV_DELAY_2, AluInp.PREV_DELAY_2)
        else:
            dp.pass_through_alu()

    def work(reseed):
        u = _wire(UopConfig(), hi=True)
        u.require_inp0 = ENABLE
        u.require_inp1 = ENABLE
        u.enable_output(OutSel.DELAY_0, OutPath.WR0_LO)  # |lo| (discarded)
        u.enable_output(OutSel.ALU_OUT, OutPath.WR0_HI)  # running max
        dps = u.datapath_config
        dps[0].enable_alu(
            AluOp.ABSOLUTE_DIFF, AluInp.PREV_DELAY_0, AluInp.PREV_DELAY_1
        )
        dps[0].pass_through_delay(3, 4)
        dps[1].enable_alu(
            AluOp.ABSOLUTE_DIFF, AluInp.PREV_DELAY_3, AluInp.PREV_DELAY_4
        )
        dps[1].enable_delay_from_src(AluInp.PREV_ALU_OUT, 0)  # lane0 <- |lo|
        dps[2].enable_alu(AluOp.MAX, AluInp.PREV_ALU_OUT, AluInp.PREV_DELAY_0)
        dps[2].pass_through_delay(0)
        if reseed:
            dps[3].enable_alu(AluOp.BYPASS, AluInp.PREV_ALU_OUT, AluInp.PREV_ALU_OUT)
            u.repeat_count = 1
            u.trigger = (Trigger.COUNT, Trigger.NONE, Trigger.NONE)
            u.next_uop = (1, 0, 0)
        else:
            dps[3].enable_alu(AluOp.MAX, AluInp.CURR_ALU_OUT, AluInp.PREV_ALU_OUT)
            u.trigger = (Trigger.SRC_TENSOR_DONE, Trigger.SUB_DIM_DONE, Trigger.NONE)
            u.next_uop = (0, 2, 0)
        dps[3].pass_through_delay(0)
        for st in range(4, 8):
            dps[st].pass_through_alu()
            dps[st].pass_through_delay(0)
        return u

    return [seed, work(False), work(True)]


class _PagedOp(_DveOp):
    """DveOp with hand-written 1x + 2x three-state uop programs."""

    def compile(self, ver):
        key = (self.name, ver)
        cached = _dve_ops._COMPILE_CACHE.get(key)
        if cached is not None:
            return cached
        spec = DveOpSpec(
            name=self.name,
            opcode=_dve_ops.get_dve_sub_opcode(self.name),
            uops=_mk_1x_uops(),
            rd1_en=True,
            uops_2x=_mk_2x_uops(),
            perf_max=1,
        )
        _dve_ops._COMPILE_CACHE[key] = spec
        return spec


def _register() -> _DveOp:
    for op in _dve_ops.OPS:
        if op.name == _PAGED_NAME:
            return op
    row = _dve_ops._CUSTOM_DVE_ROW_BASE + len(_dve_ops.OPS)
    assert row < 0x20
    op = _PagedOp(_PAGED_NAME, _PAGED_SPEC, subdim=True, uops_sha={})
    _dve_ops.OPS.append(op)
    _dve_ops.CUSTOM_DVE_SPECS[_PAGED_NAME] = _PAGED_SPEC
    _dve_ops._SUB_OPCODE_FOR_NAME[_PAGED_NAME] = row
    return op


PAGED_OP = _register()

_CACHE = {}


def _build_program(loop_n=None, perf_max=1):
    key = ("nc", loop_n, perf_max)
    if key in _CACHE:
        return _CACHE[key]
    nc = bacc.Bacc("TRN2", num_devices=NCORES)
    xs_ext = nc.declare_dram_parameter("xs", [XS_SIZE], BF16, isOutput=False)
    wr_ext = nc.declare_dram_parameter("wr", [COUT, D], BF16, isOutput=False)
    bias_ext = nc.declare_dram_parameter("bias", [1, COUT], mybir.dt.float32, isOutput=False)
    out_ext = nc.declare_dram_parameter(
        "out", [NTILES * P, COUT], mybir.dt.float32, isOutput=True
    )
    ap_cls = type(xs_ext[:].ap)

    with TileContext(nc) as tc:
        with tc.tile_pool(name="sbuf", bufs=1) as pool:
            from contextlib import nullcontext

            loop_cm = tc.For_i(0, loop_n, 1) if loop_n else nullcontext()
            with loop_cm:
                xbig = pool.tile([P, NTILES * D], BF16)
                # im2col patch loads: one strided DMA per (batch, tile)
                for b in range(B_PER):
                    for t in range(TILES_PER_BATCH):
                        idx = b * TILES_PER_BATCH + t
                        src = xs_ext[:].copy()
                        src.offset = b * XS_IMG + t * P * CIN
                        src.ap = ap_cls([[CIN, P], [WP * CIN, K], [CIN, K], [1, CIN]])
                        nc.sync.dma_start(xbig[:, idx * D : (idx + 1) * D], src)

                acc = pool.tile([P, NTILES * COUT], mybir.dt.float32)
                bias_b = pool.tile([P, COUT], mybir.dt.float32)
                nc.sync.dma_start(bias_b[:], bias_ext[0:1, :].broadcast_to([P, COUT]))

                for og in range(COUT // S):
                    wb8 = pool.tile([P, S * D], BF16, tag=f"wb{og % 2}")
                    wsrc = wr_ext[:].copy()
                    wsrc.offset = og * S * D
                    wsrc.ap = ap_cls([[0, P], [D, S], [1, D]])
                    nc.sync.dma_start(wb8[:], wsrc)
                    for idx in range(NTILES):
                        j = og * NTILES + idx
                        scr = pool.tile([P, S * D], BF16, tag=f"scr{j % 3}")
                        xin = xbig[:].copy()
                        xin.offset = xbig[:].offset + idx * D
                        xin.ap = ap_cls([[NTILES * D, P], [0, S], [1, D]])
                        r = nc.vector._custom_dve(
                            PAGED_OP,
                            out=scr[:].rearrange("p (s d) -> p s d", d=D),
                            in0=xin,
                            in1=wb8[:].rearrange("p (s d) -> p s d", d=D),
                            accum_out=None,
                        )
                        r.ins.perf_max = perf_max
                        # collect each page's final element on the Scalar engine
                        gin = scr[:].copy()
                        gin.offset = scr[:].offset + D - 1
                        gin.ap = ap_cls([[S * D, P], [D, S]])
                        col = idx * COUT + og * S
                        nc.scalar.copy(acc[:, col : col + S], gin)

                # bias add (bias repeats per tile)
                bin_ = bias_b[:].copy()
                bin_.ap = ap_cls([[COUT, P], [0, NTILES], [1, COUT]])
                nc.vector.tensor_tensor(
                    acc[:].rearrange("p (t o) -> p t o", o=COUT),
                    acc[:].rearrange("p (t o) -> p t o", o=COUT),
                    bin_,
                    mybir.AluOpType.add,
                )

                # out[(t,p), o] = acc[p, t*COUT + o]
                nc.sync.dma_start(
                    out_ext[:].rearrange("(t p) o -> p t o", p=P),
                    acc[:].rearrange("p (t o) -> p t o", o=COUT),
                )

    nc.compile()
    _CACHE[key] = nc
    return nc


def _prep_inputs(x, weights, bias):
    xp = np.pad(
        x.astype(np.float32, copy=False),
        ((0, 0), (0, 0), (PADL, PADL), (PADL, PADL)),
    )
    xcl = np.ascontiguousarray(xp.transpose(0, 2, 3, 1)).astype(ml_dtypes.bfloat16)
    wr = np.ascontiguousarray(
        weights.astype(np.float32, copy=False).transpose(0, 2, 3, 1).reshape(COUT, D)
    ).astype(ml_dtypes.bfloat16)
    bias_row = np.ascontiguousarray(bias.astype(np.float32, copy=False).reshape(1, COUT))
    in_maps = []
    for core in range(NCORES):
        sl = xcl[core * B_PER : (core + 1) * B_PER].reshape(-1)
        xs = np.zeros(XS_SIZE, dtype=ml_dtypes.bfloat16)
        xs[: sl.size] = sl
        in_maps.append({"xs": xs, "wr": wr, "bias": bias_row})
    return in_maps


def _unshard(results):
    outs = []
    for core in range(NCORES):
        r = results[core]["out"]  # [NTILES*P, COUT]
        r = r.reshape(B_PER, TILES_PER_BATCH * P, COUT)[:, :POS_PER_BATCH, :]
        r = r.reshape(B_PER, H, WP, COUT)[:, :, :W, :]
        outs.append(r.transpose(0, 3, 1, 2))  # [B_PER, COUT, H, W]
    return np.concatenate(outs, axis=0)


def kernel(x, weights, bias):
    nc = _build_program()
    in_maps = _prep_inputs(np.asarray(x), np.asarray(weights), np.asarray(bias))
    res = run_bass_kernel_spmd(nc, in_maps, core_ids=list(range(NCORES)))
    return _unshard(res.results).astype(np.float32)


# revision 7
# speedup vs baseline: 933.8008x; 1.0181x over previous
"""Trainium2 Bass kernel for nn_Dist_Conv2D (Chebyshev-distance conv).

out[b,o,h,w] = max_{c,kh,kw} |x_pad[b,c,h+kh,w+kw] - weights[o,c,kh,kw]| + bias[o]
x: [16,64,56,56] f32, weights: [128,64,3,3] f32, bias: [128,1,1] f32,
K=3, stride 1, pad 1/1 -> out [16,128,56,56] f32.

Strategy (8 NeuronCores, data-parallel over batch, 2 images per core):

- Host prep: pad x to 58x58, channels-last [b, hp, wp, c], cast bf16.
  Output positions are indexed local = h*58 + w' with w' in [0,58) — the
  two halo columns are computed and discarded — so consecutive positions
  are unit-stride in the padded image and the im2col patch load for a
  128-position tile is a single strided DMA.

- Device: one fused custom DVE instruction per (128-position tile, group
  of 8 output channels). The instruction streams [P, S=8 pages, 576]
  where in0 is the x patch tile with page stride 0 and in1 holds 8
  partition-broadcast weight rows. The body computes a running (prefix)
  maximum of |x - w| via a scan recurrence (ABSOLUTE_DIFF + MAX with
  CURR_ALU_OUT feedback); a 3-state uop FSM (seed / steady / reseed)
  restarts the recurrence at each SUB_DIM_DONE page boundary. Each
  page's final element is that (tile, o)'s complete max; the otherwise
  idle Scalar engine gathers the 8 values per instruction into the fp32
  accumulator while the DVE streams on. One DVE pass per element, no
  reduce instructions.

- A hand-authored 2x_1p micro-op program (perf slot +1, instruction
  perf_max=1) processes two packed bf16 elements per cycle: stage0 |lo|,
  stage1 |hi| via the SRC_*_HI crossbar lanes, stage2 pair max, stage3
  recurrence. All streamed operands are bf16 unit-stride innermost so
  the RTL engages 2x. Measured on HW (loop-delta method): 2.36 ms per
  kernel vs 5.45 ms for the fp32 1x un-paged variant.

- Weights are broadcast across partitions once per 8-channel group;
  x tiles stay resident in SBUF; bias is added on-device; one gather
  DMA writes [positions, channels]; host drops halo columns and
  transposes to NCHW.
"""

import numpy as np
import ml_dtypes

import concourse.bacc as bacc
import concourse.mybir as mybir
from concourse.tile import TileContext
from concourse.bass_utils import run_bass_kernel_spmd

from concourse import dve_ops as _dve_ops
from concourse.dve_ops import DveOp as _DveOp
from concourse.dve_spec import (
    Spec as _Spec,
    Src0 as _Src0,
    Src1 as _Src1,
    Bin as _Bin,
    AluOp as _SpecAluOp,
    scan as _scan,
)
from concourse.dve_uop import (
    UopConfig,
    AluOp,
    AluInp,
    InpSel,
    OutSel,
    OutPath,
    Trigger,
    DveOpSpec,
    ENABLE,
)

# ---------------------------------------------------------------------------
# Problem geometry (hardcoded for this problem instance).
# ---------------------------------------------------------------------------
B, CIN, H, W = 16, 64, 56, 56
COUT, K = 128, 3
PADL = 1  # PADDING=2 split 1/1
HP, WP = H + 2, W + 2  # 58 x 58 padded image
D = CIN * K * K  # 576, patch feature dim, ordered (kh, kw, c)
NCORES = 8
B_PER = B // NCORES  # 2 batches per core
POS_PER_BATCH = H * WP  # 3248 positions incl. 2 halo columns per row
P = 128  # partitions
TILES_PER_BATCH = -(-POS_PER_BATCH // P)  # 26
NTILES = B_PER * TILES_PER_BATCH  # 52 position tiles per core
XS_IMG = HP * WP * CIN  # elements per padded channels-last image
_XS_MAX = (B_PER - 1) * XS_IMG + (TILES_PER_BATCH * P - 1 + 2 * WP + 2) * CIN + CIN
XS_SIZE = max(B_PER * XS_IMG, _XS_MAX) + 256
BF16 = mybir.dt.bfloat16
S = 8  # output channels (pages) per DVE instruction
SCR_BUFS = 3  # scratch buffers between the DVE scan and the ACT collect

# ---------------------------------------------------------------------------
# Custom DVE op: per-page prefix-max of |in0 - in1| over [P, S, N] streams.
# Registered into concourse.dve_ops at import time (the per-NEFF DVE table
# is generated client-side from dve_ops.OPS, so runtime registration is
# visible to the compile).
# ---------------------------------------------------------------------------


def _ref_paged(in0, in1, s0, s1, imm2):
    a = in0.astype(np.float32)
    b = in1.astype(np.float32)
    return np.maximum.accumulate(np.abs(a - b), axis=-1)


_PAGED_SPEC = _Spec(
    body=_scan(_SpecAluOp.MAX, _Bin(_SpecAluOp.ABSOLUTE_DIFF, _Src0, _Src1)),
    reference=_ref_paged,
)
_PAGED_NAME = "CHEB_PAGED_SCANMAX_ANT"


def _wire(u, hi):
    # crossbar lanes (lane k>=1 feeds stage0's PREV_DELAY_{k-1})
    u.enable_input(InpSel.SRC_0, 1)
    u.enable_input(InpSel.SRC_1, 2)
    u.enable_input(InpSel.MAX_NEG, 3)
    if hi:
        u.enable_input(InpSel.SRC_0_HI, 4)
        u.enable_input(InpSel.SRC_1_HI, 5)
    return u


def _mk_1x_uops():
    # scan recurrence register = stage 1's CURR_ALU_OUT flop
    seed = _wire(UopConfig(), hi=False)
    seed.repeat_count = 1
    seed.trigger = (Trigger.COUNT, Trigger.NONE, Trigger.NONE)
    seed.next_uop = (1, 0, 0)
    seed.datapath_config[0].pass_through_alu()
    seed.datapath_config[0].pass_through_delay(2)
    seed.datapath_config[1].enable_alu(
        AluOp.BYPASS, AluInp.PREV_DELAY_2, AluInp.PREV_DELAY_2
    )
    for st in range(2, 8):
        seed.datapath_config[st].pass_through_alu()

    def work(reseed):
        u = _wire(UopConfig(), hi=False)
        u.require_inp0 = ENABLE
        u.require_inp1 = ENABLE
        u.enable_output(OutSel.ALU_OUT, OutPath.WR0_LO)
        dps = u.datapath_config
        dps[0].enable_alu(
            AluOp.ABSOLUTE_DIFF, AluInp.PREV_DELAY_0, AluInp.PREV_DELAY_1
        )
        if reseed:
            # first element of a new page: recurrence <- |elem|
            dps[1].enable_alu(AluOp.BYPASS, AluInp.PREV_ALU_OUT, AluInp.PREV_ALU_OUT)
            u.repeat_count = 1
            u.trigger = (Trigger.COUNT, Trigger.NONE, Trigger.NONE)
            u.next_uop = (1, 0, 0)
        else:
            dps[1].enable_alu(AluOp.MAX, AluInp.CURR_ALU_OUT, AluInp.PREV_ALU_OUT)
            u.trigger = (Trigger.SRC_TENSOR_DONE, Trigger.SUB_DIM_DONE, Trigger.NONE)
            u.next_uop = (0, 2, 0)
        for st in range(2, 8):
            dps[st].pass_through_alu()
        return u

    return [seed, work(False), work(True)]


def _mk_2x_uops():
    seed = _wire(UopConfig(), hi=True)
    seed.repeat_count = 1
    seed.trigger = (Trigger.COUNT, Trigger.NONE, Trigger.NONE)
    seed.next_uop = (1, 0, 0)
    for st in range(8):
        dp = seed.datapath_config[st]
        if st < 3:
            dp.pass_through_alu()
            dp.pass_through_delay(2)
        elif st == 3:
            dp.enable_alu(AluOp.BYPASS, AluInp.PREV_DELAY_2, AluInp.PREV_DELAY_2)
        else:
            dp.pass_through_alu()

    def work(reseed):
        u = _wire(UopConfig(), hi=True)
        u.require_inp0 = ENABLE
        u.require_inp1 = ENABLE
        u.enable_output(OutSel.DELAY_0, OutPath.WR0_LO)  # |lo| (discarded)
        u.enable_output(OutSel.ALU_OUT, OutPath.WR0_HI)  # running max
        dps = u.datapath_config
        dps[0].enable_alu(
            AluOp.ABSOLUTE_DIFF, AluInp.PREV_DELAY_0, AluInp.PREV_DELAY_1
        )
        dps[0].pass_through_delay(3, 4)
        dps[1].enable_alu(
            AluOp.ABSOLUTE_DIFF, AluInp.PREV_DELAY_3, AluInp.PREV_DELAY_4
        )
        dps[1].enable_delay_from_src(AluInp.PREV_ALU_OUT, 0)  # lane0 <- |lo|
        dps[2].enable_alu(AluOp.MAX, AluInp.PREV_ALU_OUT, AluInp.PREV_DELAY_0)
        dps[2].pass_through_delay(0)
        if reseed:
            dps[3].enable_alu(AluOp.BYPASS, AluInp.PREV_ALU_OUT, AluInp.PREV_ALU_OUT)
            u.repeat_count = 1
            u.trigger = (Trigger.COUNT, Trigger.NONE, Trigger.NONE)
            u.next_uop = (1, 0, 0)
        else:
            dps[3].enable_alu(AluOp.MAX, AluInp.CURR_ALU_OUT, AluInp.PREV_ALU_OUT)
            u.trigger = (Trigger.SRC_TENSOR_DONE, Trigger.SUB_DIM_DONE, Trigger.NONE)
            u.next_uop = (0, 2, 0)
        dps[3].pass_through_delay(0)
        for st in range(4, 8):
            dps[st].pass_through_alu()
            dps[st].pass_through_delay(0)
        return u

    return [seed, work(False), work(True)]


class _PagedOp(_DveOp):
    """DveOp with hand-written 1x + 2x three-state uop programs."""

    def compile(self, ver):
        key = (self.name, ver)
        cached = _dve_ops._COMPILE_CACHE.get(key)
        if cached is not None:
            return cached
        spec = DveOpSpec(
            name=self.name,
            opcode=_dve_ops.get_dve_sub_opcode(self.name),
            uops=_mk_1x_uops(),
            rd1_en=True,
            uops_2x=_mk_2x_uops(),
            perf_max=1,
        )
        _dve_ops._COMPILE_CACHE[key] = spec
        return spec


def _register() -> _DveOp:
    for op in _dve_ops.OPS:
        if op.name == _PAGED_NAME:
            return op
    row = _dve_ops._CUSTOM_DVE_ROW_BASE + len(_dve_ops.OPS)
    assert row < 0x20
    op = _PagedOp(_PAGED_NAME, _PAGED_SPEC, subdim=True, uops_sha={})
    _dve_ops.OPS.append(op)
    _dve_ops.CUSTOM_DVE_SPECS[_PAGED_NAME] = _PAGED_SPEC
    _dve_ops._SUB_OPCODE_FOR_NAME[_PAGED_NAME] = row
    return op


PAGED_OP = _register()

_CACHE = {}


def _build_program(loop_n=None, perf_max=1):
    key = ("nc", loop_n, perf_max)
    if key in _CACHE:
        return _CACHE[key]
    nc = bacc.Bacc("TRN2", num_devices=NCORES)
    xs_ext = nc.declare_dram_parameter("xs", [XS_SIZE], BF16, isOutput=False)
    wr_ext = nc.declare_dram_parameter("wr", [COUT, D], BF16, isOutput=False)
    bias_ext = nc.declare_dram_parameter("bias", [1, COUT], mybir.dt.float32, isOutput=False)
    out_ext = nc.declare_dram_parameter(
        "out", [NTILES * P, COUT], mybir.dt.float32, isOutput=True
    )
    ap_cls = type(xs_ext[:].ap)

    with TileContext(nc) as tc:
        with tc.tile_pool(name="sbuf", bufs=1) as pool:
            from contextlib import nullcontext

            loop_cm = tc.For_i(0, loop_n, 1) if loop_n else nullcontext()
            with loop_cm:
                xbig = pool.tile([P, NTILES * D], BF16)
                # im2col patch loads: one strided DMA per (batch, tile)
                for b in range(B_PER):
                    for t in range(TILES_PER_BATCH):
                        idx = b * TILES_PER_BATCH + t
                        src = xs_ext[:].copy()
                        src.offset = b * XS_IMG + t * P * CIN
                        src.ap = ap_cls([[CIN, P], [WP * CIN, K], [CIN, K], [1, CIN]])
                        nc.sync.dma_start(xbig[:, idx * D : (idx + 1) * D], src)

                acc = pool.tile([P, NTILES * COUT], mybir.dt.float32)
                bias_b = pool.tile([P, COUT], mybir.dt.float32)
                nc.sync.dma_start(bias_b[:], bias_ext[0:1, :].broadcast_to([P, COUT]))

                for og in range(COUT // S):
                    wb8 = pool.tile([P, S * D], BF16, tag=f"wb{og % 2}")
                    wsrc = wr_ext[:].copy()
                    wsrc.offset = og * S * D
                    wsrc.ap = ap_cls([[0, P], [D, S], [1, D]])
                    nc.sync.dma_start(wb8[:], wsrc)
                    for idx in range(NTILES):
                        j = og * NTILES + idx
                        scr = pool.tile([P, S * D], BF16, tag=f"scr{j % SCR_BUFS}")
                        xin = xbig[:].copy()
                        xin.offset = xbig[:].offset + idx * D
                        xin.ap = ap_cls([[NTILES * D, P], [0, S], [1, D]])
                        r = nc.vector._custom_dve(
                            PAGED_OP,
                            out=scr[:].rearrange("p (s d) -> p s d", d=D),
                            in0=xin,
                            in1=wb8[:].rearrange("p (s d) -> p s d", d=D),
                            accum_out=None,
                        )
                        r.ins.perf_max = perf_max
                        # collect each page's final element on the Scalar engine
                        gin = scr[:].copy()
                        gin.offset = scr[:].offset + D - 1
                        gin.ap = ap_cls([[S * D, P], [D, S]])
                        col = idx * COUT + og * S
                        nc.scalar.copy(acc[:, col : col + S], gin)

                # bias add (bias repeats per tile)
                bin_ = bias_b[:].copy()
                bin_.ap = ap_cls([[COUT, P], [0, NTILES], [1, COUT]])
                nc.vector.tensor_tensor(
                    acc[:].rearrange("p (t o) -> p t o", o=COUT),
                    acc[:].rearrange("p (t o) -> p t o", o=COUT),
                    bin_,
                    mybir.AluOpType.add,
                )

                # out[(t,p), o] = acc[p, t*COUT + o]
                nc.sync.dma_start(
                    out_ext[:].rearrange("(t p) o -> p t o", p=P),
                    acc[:].rearrange("p (t o) -> p t o", o=COUT),
                )

    nc.compile()
    _CACHE[key] = nc
    return nc


def _prep_inputs(x, weights, bias):
    xp = np.pad(
        x.astype(np.float32, copy=False),
        ((0, 0), (0, 0), (PADL, PADL), (PADL, PADL)),
    )
    xcl = np.ascontiguousarray(xp.transpose(0, 2, 3, 1)).astype(ml_dtypes.bfloat16)
    wr = np.ascontiguousarray(
        weights.astype(np.float32, copy=False).transpose(0, 2, 3, 1).reshape(COUT, D)
    ).astype(ml_dtypes.bfloat16)
    bias_row = np.ascontiguousarray(bias.astype(np.float32, copy=False).reshape(1, COUT))
    in_maps = []
    for core in range(NCORES):
        sl = xcl[core * B_PER : (core + 1) * B_PER].reshape(-1)
        xs = np.zeros(XS_SIZE, dtype=ml_dtypes.bfloat16)
        xs[: sl.size] = sl
        in_maps.append({"xs": xs, "wr": wr, "bias": bias_row})
    return in_maps


def _unshard(results):
    outs = []
    for core in range(NCORES):
        r = results[core]["out"]  # [NTILES*P, COUT]
        r = r.reshape(B_PER, TILES_PER_BATCH * P, COUT)[:, :POS_PER_BATCH, :]
        r = r.reshape(B_PER, H, WP, COUT)[:, :, :W, :]
        outs.append(r.transpose(0, 3, 1, 2))  # [B_PER, COUT, H, W]
    return np.concatenate(outs, axis=0)


def kernel(x, weights, bias):
    nc = _build_program()
    in_maps = _prep_inputs(np.asarray(x), np.asarray(weights), np.asarray(bias))
    res = run_bass_kernel_spmd(nc, in_maps, core_ids=list(range(NCORES)))
    return _unshard(res.results).astype(np.float32)
